# revision 4
# baseline (speedup 1.0000x reference)
"""GQA attention (B=2, S=2048, D=2048, Hq=16, Hkv=4, hd=128) on 8 TRN2 cores.

Sharding: core c = b*4 + kv handles batch b and kv-head kv (with its 4 query
heads). Each core computes its partial output (A_heads @ Wo_slice) in bf16;
the host sums the 4 partials per batch in f32 and adds the bias.

All operands bf16 (PE rate 1 cycle/row, same as f32r, but half the SBUF/DMA
footprint and 2x DVE throughput on the softmax-denominator tree). x^T stays
resident in SBUF (64KB/partition) so it is loaded exactly once.

Pass B is a slot pipeline over i-blocks (512 queries each).  Slot (h, j):
  ST  S^T tile j of head h          (PE, 512 cyc)
  exp of tile j                     (ACT, ~716ns from PSUM)
  OP  one out-proj matmul of ib-1   (PE, 512 cyc)  <- fills ACT stall
  QP  one Q-proj matmul for ib+1    (PE, 512 cyc)  <- fills ACT stall
  PV  accumulate O^T with tile j-4  (PE, 512 cyc)  <- last: max exp margin
so the PE has ~740ns of independent work per exp and the exp->PV edge has
4+ slots of slack.  Softmax denominators: bf16 tree adds on DVE + one
gpsimd partition_all_reduce (f32); normalization is folded into the
PSUM->SBUF copy of O^T.  Out-proj PSUM->SBUF copies ride on the scalar
(ACT) engine (DVE PSUM-reads are slow on HW).
"""
import sys

sys.path.insert(0, "/opt/trn_rl_repo")
import numpy as np

B, S, D = 2, 2048, 2048
HQ, HKV, HD = 16, 4, 128
G = HQ // HKV
SCALE = HD ** -0.5
P = 128
NB = 512
DC = D // P     # 16 contraction chunks
SB = S // NB    # 4 seq blocks of 512
ST = S // P     # 16 seq tiles of 128
DEPTH = 4       # PV lag behind ST/exp
OP_DELAY = 10   # slots before out-proj of ib-1 starts draining

_CACHE = {}


def _build(reps=(1, 1, 1)):
    from collections import deque
    from contextlib import ExitStack, nullcontext

    import concourse.bacc as bacc
    import concourse.bass_isa as bass_isa
    import concourse.mybir as mybir
    import concourse.tile as tile
    from concourse.masks import make_identity

    F32 = mybir.dt.float32
    BF16 = mybir.dt.bfloat16
    Exp = mybir.ActivationFunctionType.Exp
    Mult = mybir.AluOpType.mult

    nc = bacc.Bacc("TRN2", target_bir_lowering=False, debug=False)
    xT = nc.dram_tensor("xT", [D, S], BF16, kind="ExternalInput").ap()
    wq = nc.dram_tensor("wq", [D, G * HD], BF16, kind="ExternalInput").ap()
    wk = nc.dram_tensor("wk", [D, HD], BF16, kind="ExternalInput").ap()
    wv = nc.dram_tensor("wv", [D, HD], BF16, kind="ExternalInput").ap()
    wo = nc.dram_tensor("wo", [G * HD, D], BF16, kind="ExternalInput").ap()
    out = nc.dram_tensor("out", [S, D], BF16, kind="ExternalOutput").ap()

    rr = max(reps)

    with tile.TileContext(nc) as tc, ExitStack() as stk:
        persist = stk.enter_context(tc.tile_pool(name="persist", bufs=1))
        xt_sb = persist.tile([P, DC, S], BF16)
        wq_sb = persist.tile([P, DC, G * HD], BF16)
        wk_sb = persist.tile([P, DC, HD], BF16)
        wv_sb = persist.tile([P, DC, HD], BF16)
        wo_sb = persist.tile([P, G, D], BF16)
        kt_sb = persist.tile([P, S], BF16)
        v_sb = persist.tile([P, ST, HD], BF16)
        ident = persist.tile([P, P], BF16)
        make_identity(nc, ident)

        def _loop(r):
            return tc.For_i(0, r, 1) if r > 1 else nullcontext()

        # ---- pools (shared between pass A and pass B so the pass A tail
        # overlaps the QP(0) prologue) ----
        with ExitStack() as pbs:
            vt_pool = pbs.enter_context(tc.tile_pool(name="vt", bufs=4))
            qt_pool = pbs.enter_context(tc.tile_pool(name="qt", bufs=2))
            ot_pool = pbs.enter_context(tc.tile_pool(name="ot", bufs=2))
            ex_pool = pbs.enter_context(tc.tile_pool(name="ex", bufs=2))
            dn_pool = pbs.enter_context(tc.tile_pool(name="dn", bufs=2))
            st_pool = pbs.enter_context(tc.tile_pool(name="st", bufs=3))
            ps_s = pbs.enter_context(tc.tile_pool(name="ps_s", bufs=3, space="PSUM"))
            ps_o = pbs.enter_context(tc.tile_pool(name="ps_o", bufs=2, space="PSUM"))
            ps_q = pbs.enter_context(tc.tile_pool(name="ps_q", bufs=1, space="PSUM"))
            ps_p = pbs.enter_context(tc.tile_pool(name="ps_p", bufs=2, space="PSUM"))

            def qp_ops(ib, qt_dst):
                """Q-proj for i-block ib into qt_dst: 64 matmuls + 4 copies."""
                icols = slice(ib * NB, (ib + 1) * NB)
                ops = deque()
                state = {}
                for hq in range(G):
                    for c in range(DC):
                        def _qp(hq=hq, c=c):
                            if c == 0:
                                state["pq"] = ps_q.tile([P, NB], F32, name="pq", tag="pq")
                            nc.tensor.matmul(state["pq"],
                                             wq_sb[:, c, hq * HD:(hq + 1) * HD],
                                             xt_sb[:, c, icols],
                                             start=(c == 0), stop=(c == DC - 1))
                            if c == DC - 1:
                                nc.vector.tensor_copy(out=qt_dst[:, hq, :],
                                                      in_=state["pq"])
                        ops.append(_qp)
                return ops

            def op_ops(ib, ot_src):
                """Out-proj of i-block ib from ot_src: 64 matmuls (+copy+dma)."""
                ops = deque()
                state = {}
                for t in range(4):
                    it = 4 * ib + t
                    for nb in range(D // NB):
                        for ho in range(G):
                            def _op(t=t, it=it, nb=nb, ho=ho):
                                if ho == 0:
                                    state["pso"] = ps_p.tile([P, NB], F32, name="pso", tag="pso")
                                nc.tensor.matmul(state["pso"],
                                                 ot_src[:, ho, t * P:(t + 1) * P],
                                                 wo_sb[:, ho, nb * NB:(nb + 1) * NB],
                                                 start=(ho == 0), stop=(ho == G - 1))
                                if ho == G - 1:
                                    so = st_pool.tile([P, NB], BF16, name="so")
                                    nc.scalar.copy(out=so, in_=state["pso"])
                                    nc.sync.dma_start(
                                        out=out[it * P:(it + 1) * P,
                                                nb * NB:(nb + 1) * NB],
                                        in_=so)
                            ops.append(_op)
                return ops

            # ---- whole kernel body in one (optional) timing loop: DMAs,
            # pass A, pass B.  Per-rep time == one full kernel execution.
            with _loop(rr):
              # DMA order: per chunk wk/wv/xt (pass A consumes chunks as
              # they arrive), then wq (needed by the QP(0) prologue), then
              # wo (needed from ib=1 on).
              for c in range(DC):
                nc.sync.dma_start(out=wk_sb[:, c, :], in_=wk[c * P:(c + 1) * P, :])
                nc.sync.dma_start(out=wv_sb[:, c, :], in_=wv[c * P:(c + 1) * P, :])
                nc.sync.dma_start(out=xt_sb[:, c, :], in_=xT[c * P:(c + 1) * P, :])
              for c in range(DC):
                nc.sync.dma_start(out=wq_sb[:, c, :], in_=wq[c * P:(c + 1) * P, :])
              for h in range(G):
                nc.sync.dma_start(out=wo_sb[:, h, :], in_=wo[h * P:(h + 1) * P, :])

              # ---- pass A: chunk-major K^T / V^T so the PE consumes each
              # x^T chunk as its DMA lands; all 8 PSUM banks accumulate.
              pks = [ps_s.tile([P, NB], F32, name=f"pk{xb}", tag="pss")
                     for xb in range(3)]
              pks.append(ps_q.tile([P, NB], F32, name="pk3", tag="pq"))
              pvs = [ps_o.tile([P, NB], F32, name=f"pv{xb}", tag="po")
                     for xb in range(2)]
              pvs += [ps_p.tile([P, NB], F32, name=f"pv{xb}", tag="pso")
                      for xb in (2, 3)]
              for c in range(DC):
                  for xb in range(SB):
                      cols = slice(xb * NB, (xb + 1) * NB)
                      nc.tensor.matmul(pks[xb], wk_sb[:, c, :],
                                       xt_sb[:, c, cols],
                                       start=(c == 0), stop=(c == DC - 1))
                      nc.tensor.matmul(pvs[xb], wv_sb[:, c, :],
                                       xt_sb[:, c, cols],
                                       start=(c == 0), stop=(c == DC - 1))
              # kt3 first: QP(0) allocates from ps_q and needs its bank
              nc.vector.tensor_copy(out=kt_sb[:, 3 * NB:4 * NB], in_=pks[3])
              vts = []
              for xb in range(SB):
                  if xb < 3:
                      nc.vector.tensor_copy(
                          out=kt_sb[:, xb * NB:(xb + 1) * NB], in_=pks[xb])
                  vt = vt_pool.tile([P, NB], BF16, name="vt")
                  nc.vector.tensor_copy(out=vt, in_=pvs[xb])
                  vts.append(vt)

              # ---- pass B ----
              # prologue: Q-proj for ib=0 runs alone
              qt_cur = qt_pool.tile([P, G, NB], BF16, name="qt")
              for f in qp_ops(0, qt_cur):
                  f()
              # V transposes ride behind the prologue (PE idle-free: the
              # DVE copies they wait on complete during QP(0))
              for k in range(ST):
                  pt = ps_s.tile([P, P], BF16, name="pt", tag="pss")
                  nc.tensor.transpose(pt, vts[k // 4][:, (k % 4) * P:
                                                      (k % 4 + 1) * P],
                                      ident)
                  nc.vector.tensor_copy(out=v_sb[:, k, :], in_=pt)
              opq = deque()
              for ib in range(SB):
                qt_next = None
                qpq = deque()
                if ib + 1 < SB:
                    qt_next = qt_pool.tile([P, G, NB], BF16, name="qt")
                    qpq = qp_ops(ib + 1, qt_next)
                ot_ib = ot_pool.tile([P, G, NB], BF16, name="ot")
                slot = 0
                for h in range(G):
                    po = ps_o.tile([P, NB], F32, name="po", tag="po")
                    exbs = [None, None]
                    dgs = [None, None]
                    for jj in range(ST + DEPTH):
                        if jj < ST:
                            g, jo = divmod(jj, 8)
                            if jo == 0:
                                exbs[g] = ex_pool.tile([P, 8, NB], BF16, name="ex")
                            pss = ps_s.tile([P, NB], F32, name="pss", tag="pss")
                            nc.tensor.matmul(pss, kt_sb[:, jj * P:(jj + 1) * P],
                                             qt_cur[:, h, :], start=True, stop=True)
                            nc.scalar.activation(out=exbs[g][:, jo, :], in_=pss,
                                                 func=Exp, scale=SCALE)
                        if opq and slot >= OP_DELAY:
                            opq.popleft()()
                        if qpq:
                            qpq.popleft()()
                        jc = jj - DEPTH
                        if jc >= 0 and jc < ST:
                            nc.tensor.matmul(po, v_sb[:, jc, :],
                                             exbs[jc // 8][:, jc % 8, :],
                                             start=(jc == 0), stop=(jc == ST - 1))
                        if jj < ST:
                            if jo == 7:
                                exf = exbs[g]
                                t4 = dn_pool.tile([P, 4, NB], BF16, name=f"t4{g}")
                                nc.vector.tensor_add(out=t4, in0=exf[:, 0:4, :],
                                                     in1=exf[:, 4:8, :])
                                nc.vector.tensor_add(out=t4[:, 0:2, :],
                                                     in0=t4[:, 0:2, :],
                                                     in1=t4[:, 2:4, :])
                                nc.vector.tensor_add(out=t4[:, 0, :],
                                                     in0=t4[:, 0, :],
                                                     in1=t4[:, 1, :])
                                dgs[g] = t4
                        slot += 1
                    dsum = dn_pool.tile([P, NB], F32, name="dsum")
                    nc.vector.tensor_add(out=dsum, in0=dgs[0][:, 0, :],
                                         in1=dgs[1][:, 0, :])
                    dred = dn_pool.tile([P, NB], F32, name="dred")
                    nc.gpsimd.partition_all_reduce(dred, dsum, P,
                                                   bass_isa.ReduceOp.add)
                    nc.vector.reciprocal(out=dsum, in_=dred)
                    nc.vector.tensor_tensor(out=ot_ib[:, h, :], in0=po,
                                            in1=dsum, op=Mult)
                # drain any leftover interleaved work of this ib
                while opq:
                    opq.popleft()()
                while qpq:
                    qpq.popleft()()
                opq = op_ops(ib, ot_ib)
                qt_cur = qt_next
              # epilogue: out-proj of the last i-block
              while opq:
                  opq.popleft()()

    nc.compile()
    return nc


def _get_nc():
    if "nc" not in _CACHE:
        _CACHE["nc"] = _build()
    return _CACHE["nc"]


def timed_runner(reps):
    nc = _build(reps)
    return make_runner(nc)


def make_runner(nc, n_cores=8):
    """Persistent jitted SPMD runner (mirrors bass2jax.run_bass_via_pjrt's
    multi-core path, without donation so the executable can be re-invoked on
    device-resident inputs for timing)."""
    import jax
    from jax.experimental.shard_map import shard_map
    from jax.sharding import Mesh, PartitionSpec

    import concourse.mybir as mybir
    from concourse import bass2jax

    bass2jax.install_neuronx_cc_hook()
    partition_name = nc.partition_id_tensor.name if nc.partition_id_tensor else None
    in_names, out_names, out_avals, zero_shapes = [], [], [], []
    for alloc in nc.m.functions[0].allocations:
        if not isinstance(alloc, mybir.MemoryLocationSet):
            continue
        name = alloc.memorylocations[0].name
        if alloc.kind == "ExternalInput":
            if name != partition_name:
                in_names.append(name)
        elif alloc.kind == "ExternalOutput":
            out_names.append(name)
            shape = tuple(alloc.tensor_shape)
            dtype = mybir.dt.np(alloc.dtype)
            out_avals.append(jax.core.ShapedArray(shape, dtype))
            zero_shapes.append((shape, dtype))
    n_params = len(in_names)
    all_in_names = tuple(in_names + out_names)
    if partition_name is not None:
        all_in_names = all_in_names + (partition_name,)

    def _body(*args):
        operands = list(args)
        if partition_name is not None:
            operands.append(bass2jax.partition_id_tensor())
        outs = bass2jax._bass_exec_p.bind(
            *operands,
            out_avals=tuple(out_avals),
            in_names=all_in_names,
            out_names=tuple(out_names),
            lowering_input_output_aliases=(),
            sim_require_finite=True,
            sim_require_nnan=True,
            nc=nc,
        )
        return tuple(outs)

    devices = jax.devices()[:n_cores]
    mesh = Mesh(np.asarray(devices), ("core",))
    n_outs = len(out_names)
    fn = jax.jit(
        shard_map(_body, mesh=mesh,
                  in_specs=(PartitionSpec("core"),) * (n_params + n_outs),
                  out_specs=(PartitionSpec("core"),) * n_outs,
                  check_rep=False),
        keep_unused=True,
    )
    return fn, in_names, out_names, zero_shapes, mesh


def _get_runner():
    if "runner" not in _CACHE:
        _CACHE["runner"] = make_runner(_get_nc())
    return _CACHE["runner"]


def run_cores(in_maps):
    """Run the 8-core SPMD program; returns list of per-core {name: array}."""
    import jax

    fn, in_names, out_names, zero_shapes, mesh = _get_runner()
    n = len(in_maps)
    concat_in = [np.concatenate([np.asarray(in_maps[c][nm]) for c in range(n)], axis=0)
                 for nm in in_names]
    concat_zero = [np.zeros((n * s[0], *s[1:]), dt) for s, dt in zero_shapes]
    outs = fn(*concat_in, *concat_zero)
    outs = [np.asarray(o) for o in outs]
    return [
        {nm: outs[i].reshape(n, *zero_shapes[i][0])[c] for i, nm in enumerate(out_names)}
        for c in range(n)
    ]


def shard_inputs(x, Wq, Wk, Wv, Wo):
    import ml_dtypes

    BF = ml_dtypes.bfloat16
    in_maps = []
    for b in range(B):
        xTb = np.ascontiguousarray(x[b].T.astype(BF))
        for kv in range(HKV):
            in_maps.append({
                "xT": xTb,
                "wq": np.ascontiguousarray(Wq[:, kv * G * HD:(kv + 1) * G * HD].astype(BF)),
                "wk": np.ascontiguousarray(Wk[:, kv * HD:(kv + 1) * HD].astype(BF)),
                "wv": np.ascontiguousarray(Wv[:, kv * HD:(kv + 1) * HD].astype(BF)),
                "wo": np.ascontiguousarray(Wo[kv * G * HD:(kv + 1) * G * HD, :].astype(BF)),
            })
    return in_maps


def kernel(x, Wq, Wk, Wv, Wo, bo):
    x = np.asarray(x, np.float32)
    Wq = np.asarray(Wq, np.float32)
    Wk = np.asarray(Wk, np.float32)
    Wv = np.asarray(Wv, np.float32)
    Wo = np.asarray(Wo, np.float32)
    bo = np.asarray(bo, np.float32)
    results = run_cores(shard_inputs(x, Wq, Wk, Wv, Wo))
    out = np.empty((B, S, D), np.float32)
    for b in range(B):
        acc = results[4 * b]["out"].astype(np.float32)
        for kv in range(1, HKV):
            acc += results[4 * b + kv]["out"].astype(np.float32)
        out[b] = acc + bo
    return out
